# revision 1
# baseline (speedup 1.0000x reference)
"""FP8 block-wise dequant linear: out[b,s,o] = sum_i x[b,s,i] * (w[o,i]*scale[o//128,i//128]).

Sharding: 4-way over seq x 2-way over out_features across 8 NeuronCores.
Per core: x shard [512 seq, 4096 in] (bf16, host-precast), w shard
[2048 out, 4096 in] fp8, out [512, 2048] f32. All DRAM buffers are
host-staged partition-major (contraction dim on partitions, 128 KiB
contiguous per partition) so SWDGE DMAs spray across all 16 SDMA engines.

Device pipeline per core:
  - GpSimd (SWDGE) DMAs x bf16 straight into a resident SBUF tile.
  - GpSimd DMAs w fp8 slabs; VectorE dequantizes to bf16 via tensor_tensor
    with a free-dim-broadcast per-128x128-block scale operand (resident).
  - TensorE: per seq block, kb outer / out-chunk inner, so each stationary
    x-block load feeds 4 N=512 bf16 matmuls; 32 kb accumulate in 4 PSUM
    banks; VectorE evacuates; GpSimd DMAs out.

Session-2 findings (all measured on the 8-core-loaded axon trn2, For_i
loop-slope with ~16k iters + min-stat; dispatch noise is +-300 ms so short
loops are meaningless):
  - Sustained PE throughput is ~0.60-0.67 ns per moving column in every
    realistic pattern (N=64..512, chains or singles, either operand
    stationary, bank interleaving, LDW dedup) -- NOT the 0.4167 ns/col a
    2.4 GHz clock would give. The mm phase is pinned at ~160-185 us; the
    N=512 4-chain structure is as good as anything tried.
  - fp8 moving/stationary cannot help: e4m3 x quantization alone gives
    rel_err 2.65e-2 > 2e-2 gate, and DoubleRow cannot honor per-kb block
    scales inside a PSUM accumulation group anyway.
  - VectorE dequant (~68 us, broadcast-operand tensor_tensor runs at half
    the plain-copy rate regardless of scale dtype) only has a ~44 us
    refill window under the sb-major mm order => ~20 us/iter exposed.
    Fixed by unrolling 2 bodies per For_i iteration with snake (asc/desc)
    kb order: every wq slab then has a full-body window. 200 -> ~183 us.
  - Do NOT put big DMAs on the SP/ACT HWDGE rings in a For_i loop: the
    in-flight DMA blocks that engine's loop-barrier instructions (+10 us).
"""

import numpy as np
import ml_dtypes

import concourse.bacc as bacc
import concourse.mybir as mybir
from concourse.tile import TileContext
from concourse.bass_utils import run_bass_kernel_spmd


def _dedup_ldw(nc):
    """Post-compile peephole: drop an InstLdweights whose weights AP matches
    the previous kept one in the same block (and which carries no sync) —
    the PE array retains the stationary operand across matmuls, so the
    reload is pure overhead (~53 ns each at FWL rate)."""
    removed = 0
    for f in nc.m.functions:
        for bb in f.blocks:
            insts = list(bb.instructions)
            out = []
            prev_key = None
            for inst in insts:
                t = type(inst).__name__
                if t == "InstLdweights":
                    si = inst.sync_info
                    has_sync = si is not None and (bool(si.on_wait) or
                                                   bool(si.on_update))
                    key = (str(inst.ins[0]), str(inst.perf_mode),
                           str(inst.is_transpose))
                    if key == prev_key and not has_sync:
                        removed += 1
                        continue
                    prev_key = key
                elif t in ("InstMatmult", "InstEventSemaphore", "InstDrain"):
                    pass  # leaves the loaded stationary operand intact
                else:
                    prev_key = None
                out.append(inst)
            if len(out) != len(insts):
                bb.instructions = out
    return removed

SEQ, DIN, DOUT = 2048, 4096, 4096
N_CORES = 8
SEQ_SHARDS, OUT_SHARDS = 4, 2
SEQ_SH, OUT_SH = SEQ // SEQ_SHARDS, DOUT // OUT_SHARDS  # 1024, 1024
P = 128
NKB = DIN // P            # 32 contraction blocks
NOB = OUT_SH // P         # 8 out blocks per core
NMM = 512                 # matmul moving free dim (PSUM one-bank limit)
NOC = OUT_SH // NMM       # 2 out chunks per core
NSB = SEQ_SH // P         # 8 seq blocks per core


XPIECES = 2   # x DMA transfers (bf16, host-precast), 4 MiB each
WCHUNK = 8    # kb slabs per w DMA (2 MiB fp8 transfers; halves SWDGE fixed
              # cost, 206.0 vs 207.1 us/exec interleaved, staging still fits)
DMA_ENGINE = "gpsimd"


def _dma(nc):
    return getattr(nc, DMA_ENGINE)


SCALE_BF16 = False  # bf16 scale operand measured no faster than f32; keep f32


def alloc_bufs(nc, pools, io):
    dt = mybir.dt
    persist, wf_pool, ob_pool, ps_pool = pools
    xt, wt, sc, out = io
    sc_sb = persist.tile([P, NKB * NOB], dt.float32, tag="sc")
    nc.sync.dma_start(sc_sb[:], sc[:])  # gpsimd variant measured ~1us slower
    if SCALE_BF16:
        sc_bf = persist.tile([P, NKB * NOB], dt.bfloat16, tag="scb")
        nc.vector.tensor_copy(sc_bf[:], sc_sb[:])
        sc_sb = sc_bf
    xb_all = persist.tile([P, NKB * SEQ_SH], dt.bfloat16, tag="xb")
    wq_all = persist.tile([P, NKB * OUT_SH], dt.bfloat16, tag="wq")
    return sc_sb, xb_all, wq_all


def emit_load(nc, pools, io, nkb=NKB, load_x=True, load_w=True, bufs=None,
              desc=False):
    """x arrives bf16 (host-precast, same RNE rounding the on-chip cast would
    do) and is DMA'd straight into the resident activation tile. w arrives
    fp8, staged, and dequantized to bf16 by VectorE tensor_tensor with a
    free-dim-broadcast per-block scale. All DRAM is host-staged
    partition-major so transfers spray across all 16 SDMA engines.

    desc=True loads/dequantizes kb chunks in descending order, to pair with
    a descending-order consumer (snake unrolling)."""
    dt = mybir.dt
    persist, wf_pool, ob_pool, ps_pool = pools
    xt, wt, sc, out = io

    if bufs is None:
        bufs = alloc_bufs(nc, pools, io)
    sc_sb, xb_all, wq_all = bufs

    wq = [None] * nkb
    xb = []
    assert nkb % XPIECES == 0
    xstep = nkb // XPIECES

    def issue_w(kb0):
        nb = min(WCHUNK, nkb - kb0)
        wf = wf_pool.tile([P, WCHUNK * OUT_SH], dt.float8e4, tag="wf")
        _dma(nc).dma_start(
            wf[:, :nb * OUT_SH],
            wt[:, kb0 * OUT_SH:(kb0 + nb) * OUT_SH])
        js = range(nb - 1, -1, -1) if desc else range(nb)
        for j in js:
            kb = kb0 + j
            s_b = (sc_sb[:, kb * NOB:(kb + 1) * NOB]
                   .unsqueeze(2).broadcast_to([P, NOB, P]))
            wslab = wq_all[:, kb * OUT_SH:(kb + 1) * OUT_SH]
            nc.vector.tensor_mul(
                wslab.rearrange("p (b i) -> p b i", b=NOB),
                wf[:, j * OUT_SH:(j + 1) * OUT_SH]
                .rearrange("p (b i) -> p b i", b=NOB),
                s_b,
            )
            wq[kb] = wslab

    chunk0s = list(range(0, nkb, WCHUNK))
    if desc:
        chunk0s = chunk0s[::-1]
    xorder = range(XPIECES - 1, -1, -1) if desc else range(XPIECES)

    # The first matmuls need dequantized w, so the first two w chunks jump
    # the SWDGE FIFO ahead of the x pieces (their staging slots are always
    # free, so no head-of-line blocking); remaining chunks follow.
    if load_w:
        for kb0 in chunk0s[:2]:
            issue_w(kb0)
    if load_x:
        for i in xorder:
            lo, hi = i * xstep * SEQ_SH, (i + 1) * xstep * SEQ_SH
            _dma(nc).dma_start(xb_all[:, lo:hi], xt[:, lo:hi])
    if load_w:
        for kb0 in chunk0s[2:]:
            issue_w(kb0)
    else:
        wq = [wq_all[:, kb * OUT_SH:(kb + 1) * OUT_SH] for kb in range(nkb)]
    for kb in range(nkb):
        xb.append(xb_all[:, kb * SEQ_SH:(kb + 1) * SEQ_SH])
    return xb, wq


MMN = 512  # matmul moving free dim; sub-bank values pack 512//MMN chains/bank
OUT_BATCH = 2  # PSUM banks per output DMA; 2 measured better than 4
               # (interleaved A/B: 227.5 vs 239.1 us/exec on the hot device)


def emit_mm(nc, pools, io, xb, wq, nkb=NKB, nsb=NSB, noc=NOC, desc=False):
    """Accumulating matmuls + PSUM evacuation + output DMA.

    Loop order: for each seq block, kb is the outer loop and the out-chunks
    are inner, so consecutive matmuls share the stationary operand xb[kb][sb].
    For MMN < 512, 512//MMN chains pack into one PSUM bank; start=True clears
    has_written for the whole bank, so only the first chain of each bank
    starts the group — the other chains' first write lands on cleared bits
    (flags=0 overwrite+set-bit).

    out is host-staged partition-major: out[p, sb*OUT_SH+o] = y[sb*128+p, o]."""
    dt = mybir.dt
    persist, wf_pool, ob_pool, ps_pool = pools
    xt, wt, sc, out = io
    n = MMN
    nch = OUT_SH // n            # moving chunks per sb (covering all banks)
    per_bank = max(1, 512 // n)  # chains per bank
    nbank = OUT_SH // 512        # banks per sb == NOC
    for sb in range(nsb):
        pss = []
        for b in range(nbank):
            ps = ps_pool.tile([P, 512], dt.float32, tag="ps")
            pss.append(ps)
        kbs = range(nkb - 1, -1, -1) if desc else range(nkb)
        for i, kb in enumerate(kbs):
            lhs = xb[kb][:, sb * P:(sb + 1) * P]
            for oc in range(nch):
                bank = pss[oc // per_bank]
                lo = (oc % per_bank) * n
                nc.tensor.matmul(
                    bank[:, lo:lo + n],
                    lhs,
                    wq[kb][:, oc * n:(oc + 1) * n],
                    start=(i == 0 and oc % per_bank == 0),
                    stop=(i == nkb - 1),
                    skip_group_check=(per_bank > 1),
                )
        for half in range(0, nbank, OUT_BATCH):
            nb2 = min(OUT_BATCH, nbank - half)
            ob = ob_pool.tile([P, OUT_BATCH * 512], dt.float32, tag="ob")
            for j in range(nb2):
                nc.vector.tensor_copy(ob[:, j * 512:(j + 1) * 512],
                                      pss[half + j][:])
            _dma(nc).dma_start(
                out[:, sb * OUT_SH + half * 512:
                       sb * OUT_SH + (half + nb2) * 512],
                ob[:, :nb2 * 512])


def emit_body(nc, tc, pools, io, it, nkb=NKB, nsb=NSB, noc=NOC,
              do_mm=True, do_load=True, desc=False):
    dt = mybir.dt
    persist, wf_pool, ob_pool, ps_pool = pools
    xt, wt, sc, out = io
    if do_load:
        xb, wq = emit_load(nc, pools, io, nkb=nkb, desc=desc)
    else:
        xb_all = persist.tile([P, NKB * SEQ_SH], dt.bfloat16, tag="xb")
        wq_all = persist.tile([P, NKB * OUT_SH], dt.bfloat16, tag="wq")
        xb = [xb_all[:, kb * SEQ_SH:(kb + 1) * SEQ_SH] for kb in range(nkb)]
        wq = [wq_all[:, kb * OUT_SH:(kb + 1) * OUT_SH] for kb in range(nkb)]
    if not do_mm:
        ob = ob_pool.tile([P, NMM], dt.float32, tag="ob")
        nc.vector.tensor_copy(ob[:], wq[0][:, 0:NMM])
        nc.sync.dma_start(out[0:P, 0:NMM], ob[:])
        return
    emit_mm(nc, pools, io, xb, wq, nkb=nkb, nsb=nsb, noc=noc, desc=desc)


def build_nc(iters=1, loop=None, unroll=1, **kw):
    nc = bacc.Bacc(None, target_bir_lowering=False)
    xt = nc.dram_tensor("xt", [P, NKB * SEQ_SH], mybir.dt.bfloat16, kind="ExternalInput")
    wt = nc.dram_tensor("wt", [P, NKB * OUT_SH], mybir.dt.float8e4, kind="ExternalInput")
    sc = nc.dram_tensor("sc", [P, NKB * NOB], mybir.dt.float32, kind="ExternalInput")
    out = nc.dram_tensor("out", [P, NSB * OUT_SH], mybir.dt.float32,
                         kind="ExternalOutput")
    io = (xt, wt, sc, out)

    with TileContext(nc) as tc:
        with (
            tc.tile_pool(name="persist", bufs=1) as persist,
            tc.tile_pool(name="wf", bufs=2) as wf_pool,
            tc.tile_pool(name="ob", bufs=3) as ob_pool,
            tc.tile_pool(name="ps", bufs=4096 // NMM, space="PSUM") as ps_pool,
        ):
            pools = (persist, wf_pool, ob_pool, ps_pool)
            if loop is not None:
                phase = kw.pop("loop_phase", "all")
                if phase == "mm":
                    xb, wq = emit_load(nc, pools, io)
                    with tc.For_i(0, loop, 1):
                        emit_mm(nc, pools, io, xb, wq)
                elif phase in ("all_nox", "all_now"):
                    # load one operand once outside; loop reloads the other
                    bufs = alloc_bufs(nc, pools, io)
                    xb, wq = emit_load(nc, pools, io, bufs=bufs)
                    with tc.For_i(0, loop, 1):
                        xb2, wq2 = emit_load(nc, pools, io, bufs=bufs,
                                             load_x=(phase == "all_now"),
                                             load_w=(phase == "all_nox"))
                        emit_mm(nc, pools, io, xb2, wq2)
                elif phase == "load":
                    with tc.For_i(0, loop, 1):
                        emit_body(nc, tc, pools, io, 0, do_mm=False, **kw)
                elif phase == "nop":
                    bufs = alloc_bufs(nc, pools, io)
                    xb, wq = emit_load(nc, pools, io, bufs=bufs)
                    with tc.For_i(0, loop, 1):
                        ob = ob_pool.tile([P, NMM], mybir.dt.float32, tag="ob")
                        nc.vector.tensor_copy(
                            ob[:], wq[0][:, :2 * NMM].bitcast(mybir.dt.float32))
                        nc.sync.dma_start(out[0:P, 0:NMM], ob[:])
                else:
                    with tc.For_i(0, loop, 1):
                        for u in range(unroll):
                            # snake: alternate kb direction so each wq slab's
                            # refill window spans ~a full body execution
                            emit_body(nc, tc, pools, io, 0,
                                      desc=(u % 2 == 1), **kw)
            else:
                for it in range(iters):
                    emit_body(nc, tc, pools, io, it, **kw)
    nc.compile()
    # _dedup_ldw(nc) measured as a machine-level no-op (walrus self-loads
    # the stationary operand per matmul regardless); left off for safety.
    return nc


def shard_inputs(x, weight, weight_scale_inv):
    """Host staging, partition-major per core:
       xt[p, kb*SEQ_SH+f] = x[0][si*SEQ_SH+f, kb*128+p]
       wt[p, kb*OUT_SH+o] = weight[oi*OUT_SH+o, kb*128+p]
       sc[p, kb*NOB+ob]   = weight_scale_inv[oi*NOB+ob, kb]"""
    x = np.asarray(x)
    weight = np.asarray(weight)
    scale = np.asarray(weight_scale_inv, dtype=np.float32)
    w8 = weight.view(np.uint8)

    in_maps = []
    x_dev = {}
    w_dev = {}
    for c in range(N_CORES):
        si, oi = c % SEQ_SHARDS, c // SEQ_SHARDS
        if si not in x_dev:
            xs = np.asarray(x[0][si * SEQ_SH:(si + 1) * SEQ_SH, :],
                            dtype=np.float32).astype(ml_dtypes.bfloat16)
            x_dev[si] = np.ascontiguousarray(
                xs.T.reshape(NKB, P, SEQ_SH).transpose(1, 0, 2)
            ).reshape(P, NKB * SEQ_SH)
        if oi not in w_dev:
            ws = w8[oi * OUT_SH:(oi + 1) * OUT_SH, :]
            w_dev[oi] = np.ascontiguousarray(
                ws.T.reshape(NKB, P, OUT_SH).transpose(1, 0, 2)
            ).reshape(P, NKB * OUT_SH).view(ml_dtypes.float8_e4m3)
        sc_core = scale.T[:, oi * NOB:(oi + 1) * NOB]        # [NKB, NOB]
        sc = np.ascontiguousarray(
            np.broadcast_to(sc_core.reshape(1, NKB * NOB), (P, NKB * NOB)))
        in_maps.append({"xt": x_dev[si], "wt": w_dev[oi], "sc": sc})
    return in_maps


def unshard_output(results):
    out = np.empty((1, SEQ, DOUT), dtype=np.float32)
    for c in range(N_CORES):
        si, oi = c % SEQ_SHARDS, c // SEQ_SHARDS
        o = results[c]["out"].reshape(P, NSB, OUT_SH).transpose(1, 0, 2)
        out[0, si * SEQ_SH:(si + 1) * SEQ_SH,
            oi * OUT_SH:(oi + 1) * OUT_SH] = o.reshape(SEQ_SH, OUT_SH)
    return out


_NC_CACHE = {}


def _run_spmd(nc, in_maps, tries=3):
    """The axon-tunneled device occasionally faults with
    NRT_EXEC_UNIT_UNRECOVERABLE, which poisons the whole PJRT client —
    reset jax backends before retrying."""
    import time as _time
    last = None
    for t in range(tries):
        try:
            return run_bass_kernel_spmd(nc, in_maps, core_ids=list(range(N_CORES)))
        except Exception as e:  # noqa: BLE001
            last = e
            _time.sleep(2.0)
            try:
                import jax as _jax
                _jax.clear_backends()
            except Exception:  # noqa: BLE001
                pass
    raise last


def kernel(x, weight, weight_scale_inv):
    if "nc" not in _NC_CACHE:
        _NC_CACHE["nc"] = build_nc()
    nc = _NC_CACHE["nc"]
    in_maps = shard_inputs(x, weight, weight_scale_inv)
    res = _run_spmd(nc, in_maps)
    return unshard_output(res.results)

